# revision 1
# baseline (speedup 1.0000x reference)
"""Multi-head attention (B=2, S=2048, D=768, H=16, dk=48) on 8 TRN2 NeuronCores.

Sharding: core c = (batch b = c//4, head-group g = c%4 of 4 heads).
Each core computes Q/K/V projections for its 4 heads, full attention over
S=2048, and a partial output projection (contribution of its heads).
Host sums the 4 partials per batch and adds the analytically-folded biases
(softmax rows sum to 1, so the V-bias contributes Wo @ bv to every row).

Device-side structure (all matmuls bf16, fp32 PSUM accumulation):
- x is fed transposed+bf16 (xT [768, 2048]); weights pre-sliced, padded and
  transposed on the host. Wq/bq pre-scaled by 1/sqrt(dk).
- Q/K projected into head-transposed layout [dk, S], 2 head-pairs per core,
  each pair [2*64, S] with a head per 64-partition strip (48 real rows).
- Attention runs per (sq-quarter of 512, pair, sk-tile): scoresT [sk, sq]
  via two concurrent row-strip matmuls (tile_position (0,0)/(64,0), K=48),
  both heads' scores in one [128, 1024] PSUM tile; one Exp per step on the
  scalar engine (the kernel's bottleneck: ~1.4us per [128,1024] op).
- Softmax denominator: a ones-column smuggled into V at cols 48/112 of each
  pair (V's bias is folded out on the host); ctx accumulates transposed in
  two single-bank PSUM tiles (one per head side -> no bank sharing, the
  col-strip matmuls run concurrently).
- Normalization: rank-2 broadcast matmul (ones2 lhsT, fp32r) replicates the
  1/denom rows; fused into the ctx eviction (tensor_tensor multiply) which
  writes bf16 ctxT -- exactly the lhsT the output projection needs.
- PSUM budget: scores 2x[128,1024] (4 banks) + ctx 2x[128,512] (2) +
  aux pool 2x[128,512] (2) for Q/K/V-proj, bcast and out-proj chunks.
"""
import os
import sys
import numpy as np
import ml_dtypes

for _p in ("/opt/trn_rl_repo", "/opt/pypackages"):
    if os.path.isdir(_p) and _p not in sys.path:
        sys.path.append(_p)

import concourse.bacc as bacc
import concourse.mybir as mybir
import concourse.tile as tile
from concourse.bass_utils import run_bass_kernel_spmd

F32 = mybir.dt.float32
F32R = mybir.dt.float32r
BF16 = mybir.dt.bfloat16
NPBF16 = ml_dtypes.bfloat16

B = 2
S = 2048
D = 768
H = 16
DK = 48
HPC = 4            # heads per core
NPAIR = 2          # head pairs per core
E = NPAIR * 128    # padded per-core head dim (4 heads x 64)
KT = D // 128      # 6 contraction tiles for projections
ST = S // 128      # 16 s-tiles
NQ = 4             # sq quarters
QW = S // NQ       # 512
NCORES = 8

_PROGRAM = None


def _build_program(variant="full"):
    nc = bacc.Bacc("TRN2", target_bir_lowering=False, debug=False)

    xT = nc.dram_tensor("xT", [D, S], BF16, kind="ExternalInput")
    wq = nc.dram_tensor("wq", [D, E], BF16, kind="ExternalInput")
    wk = nc.dram_tensor("wk", [D, E], BF16, kind="ExternalInput")
    wv = nc.dram_tensor("wv", [D, E], BF16, kind="ExternalInput")
    wo = nc.dram_tensor("wo", [E, D], BF16, kind="ExternalInput")
    bq = nc.dram_tensor("bq", [E], F32, kind="ExternalInput")
    bk = nc.dram_tensor("bk", [E], F32, kind="ExternalInput")
    ones2 = nc.dram_tensor("ones2", [2, 128], F32R, kind="ExternalInput")
    out = nc.dram_tensor("out", [S, D], F32, kind="ExternalOutput")

    EXPF = mybir.ActivationFunctionType.Exp

    with tile.TileContext(nc) as tc:
        with (
            tc.tile_pool(name="xw", bufs=1) as xw,          # x + weights
            tc.tile_pool(name="qkv", bufs=1) as qkv,        # qT/kT/v/ctxT
            tc.tile_pool(name="expp", bufs=6) as expp,      # exp tiles
            tc.tile_pool(name="outp", bufs=4) as outp,      # ctxu + out staging
            tc.tile_pool(name="misc", bufs=4) as misc,      # denom/recip
            tc.tile_pool(name="ps_sc", bufs=2, space="PSUM") as ps_sc,   # 4 banks
            tc.tile_pool(name="ps_ctx", bufs=1, space="PSUM") as ps_ctx,  # 2 banks
            tc.tile_pool(name="ps_aux", bufs=2, space="PSUM") as ps_aux,  # 2 banks
        ):
            # ---------- input DMAs ----------
            xT_sb = []
            for k in range(KT):
                t = xw.tile([128, S], BF16, name=f"xT_sb{k}", tag=f"xT_sb{k}")
                # chunked so consumers' deps resolve per 512-column slice
                for c in range(4):
                    nc.sync.dma_start(
                        out=t[:, 512 * c:512 * (c + 1)],
                        in_=xT[128 * k:128 * (k + 1), 512 * c:512 * (c + 1)])
                xT_sb.append(t)

            w_sb = {}
            for nm, dram in (("wk", wk), ("wq", wq), ("wv", wv)):
                tiles = []
                for k in range(KT):
                    t = xw.tile([128, E], BF16, name=f"{nm}_sb{k}", tag=f"{nm}_sb{k}")
                    nc.sync.dma_start(out=t[:], in_=dram[128 * k:128 * (k + 1), :])
                    tiles.append(t)
                w_sb[nm] = tiles

            wo_sb = []
            for k in range(NPAIR):
                t = xw.tile([128, D], BF16, name=f"wo_sb{k}", tag=f"wo_sb{k}")
                nc.sync.dma_start(out=t[:], in_=wo[128 * k:128 * (k + 1), :])
                wo_sb.append(t)

            bias_sb = {}
            for nm, dram in (("bq", bq), ("bk", bk)):
                t = xw.tile([128, NPAIR], F32, name=f"{nm}_sb", tag=f"{nm}_sb")
                nc.sync.dma_start(out=t[:], in_=dram.rearrange("(t p) -> p t", p=128))
                bias_sb[nm] = t

            ones_sb = xw.tile([2, 128], F32R, name="ones_sb", tag="ones_sb")
            nc.sync.dma_start(out=ones_sb[:], in_=ones2[:])

            if variant == "dma":
                junk = outp.tile([128, D], F32, name="junk", tag="o_sb")
                with nc.allow_low_precision(reason="bench"):
                    nc.vector.tensor_copy(junk[:, 0:S // 4], xT_sb[0][:, 0:S // 4].bitcast(BF16))
                    for k in range(1, KT):
                        nc.vector.tensor_copy(junk[:, 0:8], xT_sb[k][:, 0:8])
                    for nm2 in ("wk", "wq", "wv"):
                        for k in range(KT):
                            nc.vector.tensor_copy(junk[:, 0:8], w_sb[nm2][k][:, 0:8])
                    for k in range(NPAIR):
                        nc.vector.tensor_copy(junk[:, 0:8], wo_sb[k][:, 0:8])
                for st in range(ST):
                    nc.sync.dma_start(out=out[128 * st:128 * (st + 1), :],
                                      in_=junk[:])

            # ---------- persistent activations (bf16) ----------
            qT_sb = [qkv.tile([128, S], BF16, name=f"qT_sb{p}", tag=f"qT_sb{p}")
                     for p in range(NPAIR)]
            kT_sb = [qkv.tile([128, S], BF16, name=f"kT_sb{p}", tag=f"kT_sb{p}")
                     for p in range(NPAIR)]
            v_bf = [qkv.tile([128, E], BF16, name=f"v_bf{st}", tag=f"v_bf{st}")
                    for st in range(ST)]
            ctxT_sb = [qkv.tile([128, S], BF16, name=f"ctxT_sb{p}", tag=f"ctxT_sb{p}")
                       for p in range(NPAIR)]

            def emit_qk_proj(nm, t, c):
                dst = kT_sb if nm == "wk" else qT_sb
                bias = "bk" if nm == "wk" else "bq"
                ps = ps_aux.tile([128, 512], F32, name=f"ps_{nm}{t}_{c}",
                                 tag="ps_aux")
                for k in range(KT):
                    nc.tensor.matmul(
                        ps[:],
                        lhsT=w_sb[nm][k][:, 128 * t:128 * (t + 1)],
                        rhs=xT_sb[k][:, 512 * c:512 * (c + 1)],
                        start=(k == 0), stop=(k == KT - 1),
                    )
                with nc.allow_low_precision(reason="bf16 q/k"):
                    nc.vector.tensor_scalar_add(
                        dst[t][:, 512 * c:512 * (c + 1)], ps[:],
                        bias_sb[bias][:, t:t + 1])

            def emit_v_proj(st):
                ps = ps_aux.tile([128, 512], F32, name=f"ps_v{st}", tag="ps_aux")
                psv = ps[:, 0:E]
                for k in range(KT):
                    nc.tensor.matmul(
                        psv,
                        lhsT=xT_sb[k][:, 128 * st:128 * (st + 1)],
                        rhs=w_sb["wv"][k][:],
                        start=(k == 0), stop=(k == KT - 1),
                    )
                with nc.allow_low_precision(reason="probs@v in bf16"):
                    nc.vector.tensor_copy(v_bf[st][:], psv)
                for j in range(HPC):
                    nc.vector.memset(v_bf[st][:, 64 * j + 48:64 * j + 49], 1.0)

            if variant == "proj":
                for t in range(NPAIR):
                    for c in range(4):
                        emit_qk_proj("wk", t, c)
                        emit_qk_proj("wq", t, c)
                for st in range(ST):
                    emit_v_proj(st)
                with nc.allow_low_precision(reason="bench"):
                    for st in range(ST):
                        o_sb = outp.tile([128, D], F32, name=f"o_sb{st}",
                                         tag="o_sb")
                        nc.vector.tensor_copy(o_sb[:, 0:E], v_bf[st][:])
                        nc.vector.tensor_copy(o_sb[:, 0:D],
                                              qT_sb[0][:, 0:D])
                        nc.sync.dma_start(out=out[128 * st:128 * (st + 1), :],
                                          in_=o_sb[:])

            # Prologue: only what step (q0, p0, sk0) needs.
            if variant in ("full", "attn"):
                emit_qk_proj("wk", 0, 0)
                emit_qk_proj("wq", 0, 0)

            # Deferred projection chunks: (quarter, pair) -> {sk: [(nm,t,c)]}
            deferred = {
                (0, 0): {2: [("wk", 0, 1)], 5: [("wk", 0, 2)], 8: [("wk", 0, 3)],
                         12: [("wk", 1, 0)], 14: [("wq", 1, 0)]},
                (0, 1): {1: [("wk", 1, 1)], 5: [("wk", 1, 2)],
                         8: [("wk", 1, 3)], 11: [("wq", 0, 1)],
                         13: [("wq", 1, 1)]},
                (1, 1): {2: [("wq", 0, 2)], 8: [("wq", 1, 2)]},
                (2, 1): {2: [("wq", 0, 3)], 8: [("wq", 1, 3)]},
            }

            # ---------- attention + output projection ----------
            for q in range(NQ if variant in ("full", "attn") else 0):
                q0 = q * QW
                for pair in range(NPAIR):
                    ctx_ps = [ps_ctx.tile([128, QW], F32,
                                          name=f"ctx{q}_{pair}_{s}",
                                          tag=f"ps_ctx{s}")
                              for s in range(2)]
                    dmap = deferred.get((q, pair), {})
                    for sk in range(ST):
                        # scores + exp issue FIRST so the scalar engine is
                        # never delayed behind projection work on the PE;
                        # V/proj/ctx then fill the exp window.
                        sc = ps_sc.tile([128, 1024], F32,
                                        name=f"sc{q}_{pair}_{sk}", tag="ps_sc")
                        for side in range(2):
                            r0 = 64 * side
                            nc.tensor.matmul(
                                sc[:, 512 * side:512 * (side + 1)],
                                lhsT=kT_sb[pair][r0:r0 + DK,
                                                 128 * sk:128 * (sk + 1)],
                                rhs=qT_sb[pair][r0:r0 + DK, q0:q0 + QW],
                                start=True, stop=True,
                                tile_position=(r0, 0),
                            )
                        ex = expp.tile([128, 1024], BF16,
                                       name=f"ex{q}_{pair}_{sk}", tag="expp")
                        with nc.allow_low_precision(reason="probs in bf16"):
                            nc.scalar.activation(ex[:], sc[:], EXPF)
                        for nm, t, c in dmap.get(sk, ()):
                            emit_qk_proj(nm, t, c)
                        if q == 0 and pair == 0:
                            emit_v_proj(sk)
                        # ctx accumulation, one psum tensor per head side
                        for side in range(2):
                            nc.tensor.matmul(
                                ctx_ps[side][64 * side:64 * side + 64, :],
                                lhsT=v_bf[sk][:, 128 * pair + 64 * side:
                                              128 * pair + 64 * side + 64],
                                rhs=ex[:, 512 * side:512 * (side + 1)],
                                start=(sk == 0), stop=(sk == ST - 1),
                                tile_position=(0, 64 * side),
                                skip_group_check=True,
                            )

                    # ---- normalize + evict ctx for this (quarter, pair) ----
                    ctxu = outp.tile([128, QW], F32, name=f"ctxu{q}_{pair}",
                                     tag="ctxu")
                    nc.vector.tensor_copy(ctxu[0:64, :], ctx_ps[0][0:64, :])
                    nc.vector.tensor_copy(ctxu[64:128, :], ctx_ps[1][64:128, :])
                    den = misc.tile([2, QW], F32, name=f"den{q}_{pair}", tag="den")
                    nc.sync.dma_start(out=den[0:1, :], in_=ctxu[48:49, :])
                    nc.sync.dma_start(out=den[1:2, :], in_=ctxu[112:113, :])
                    rec = misc.tile([2, QW], F32R, name=f"rec{q}_{pair}", tag="rec")
                    with nc.allow_low_precision(reason="fp32r for bcast matmul"):
                        nc.vector.reciprocal(rec[:], den[:])
                    bc_ps = ps_aux.tile([128, 512], F32, name=f"bc{q}_{pair}",
                                        tag="ps_aux")
                    nc.tensor.matmul(bc_ps[:], lhsT=ones_sb[:], rhs=rec[:],
                                     start=True, stop=True)
                    with nc.allow_low_precision(reason="bf16 ctxT"):
                        nc.vector.tensor_mul(
                            ctxT_sb[pair][:, q0:q0 + QW], ctxu[:], bc_ps[:])

                # ---- output projection for this quarter's s-tiles ----
                if variant == "attn":
                    with nc.allow_low_precision(reason="bench"):
                        for sti in range(QW // 128):
                            st = q * (QW // 128) + sti
                            o_sb = outp.tile([128, D], F32, name=f"o_sb{st}",
                                             tag="o_sb")
                            nc.vector.tensor_copy(o_sb[:, 0:D],
                                                  ctxT_sb[st % 2][:, 0:D])
                            nc.sync.dma_start(out=out[128 * st:128 * (st + 1), :],
                                              in_=o_sb[:])
                    continue
                for sti in range(QW // 128):
                    st = q * (QW // 128) + sti
                    o_sb = outp.tile([128, D], F32, name=f"o_sb{st}", tag="o_sb")
                    for c0, c1 in ((0, 512), (512, D)):
                        ps = ps_aux.tile([128, 512], F32, name=f"ps_o{st}_{c0}",
                                         tag="ps_aux")
                        pso = ps[:, 0:c1 - c0]
                        for k in range(NPAIR):
                            nc.tensor.matmul(
                                pso,
                                lhsT=ctxT_sb[k][:, 128 * st:128 * (st + 1)],
                                rhs=wo_sb[k][:, c0:c1],
                                start=(k == 0), stop=(k == NPAIR - 1),
                            )
                        nc.vector.tensor_copy(o_sb[:, c0:c1], pso)
                        nc.sync.dma_start(
                            out=out[128 * st:128 * (st + 1), c0:c1],
                            in_=o_sb[:, c0:c1])

    nc.compile()
    return nc


def _prep_core_inputs(core, Wq, bq, Wk, bk, Wv):
    b, g = divmod(core, HPC)
    scale = 1.0 / np.sqrt(np.float32(DK))

    def pad_w(W, s):
        wp = np.zeros((D, E), np.float32)
        for j in range(HPC):
            h = HPC * g + j
            wp[:, 64 * j:64 * j + DK] = W[DK * h:DK * (h + 1), :].T * s
        return wp.astype(NPBF16)

    def pad_b(vec, s):
        bp = np.zeros((E,), np.float32)
        for j in range(HPC):
            h = HPC * g + j
            bp[64 * j:64 * j + DK] = vec[DK * h:DK * (h + 1)] * s
        return bp

    return b, {
        "wq": pad_w(Wq, scale),
        "bq": pad_b(bq, scale),
        "wk": pad_w(Wk, 1.0),
        "bk": pad_b(bk, 1.0),
        "wv": pad_w(Wv, 1.0),
    }


def _build_in_maps(x, Wq, bq, Wk, bk, Wv, Wo):
    ones2 = np.zeros((2, 128), np.float32)
    ones2[0, 0:64] = 1.0
    ones2[1, 64:128] = 1.0

    xT = [np.ascontiguousarray(x[b].T).astype(NPBF16) for b in range(B)]

    in_maps = []
    for core in range(NCORES):
        b, wmap = _prep_core_inputs(core, Wq, bq, Wk, bk, Wv)
        g = core % HPC
        wo_pad = np.zeros((E, D), np.float32)
        for j in range(HPC):
            h = HPC * g + j
            wo_pad[64 * j:64 * j + DK, :] = Wo[:, DK * h:DK * (h + 1)].T
        in_maps.append({
            "xT": xT[b],
            "wo": wo_pad.astype(NPBF16),
            "ones2": ones2,
            **wmap,
        })
    return in_maps


def _postprocess(results, Wo, bv, bo):
    const = (Wo @ bv + bo).astype(np.float32)  # folded V-bias + out bias
    out = np.empty((B, S, D), np.float32)
    for b in range(B):
        acc = results[HPC * b]["out"].astype(np.float32).copy()
        for g in range(1, HPC):
            acc += results[HPC * b + g]["out"]
        out[b] = acc + const
    return out


def get_program():
    global _PROGRAM
    if _PROGRAM is None:
        _PROGRAM = _build_program()
    return _PROGRAM


def kernel(x, Wq, bq, Wk, bk, Wv, bv, Wo, bo):
    x = np.asarray(x, np.float32)
    Wq, bq = np.asarray(Wq, np.float32), np.asarray(bq, np.float32)
    Wk, bk = np.asarray(Wk, np.float32), np.asarray(bk, np.float32)
    Wv, bv = np.asarray(Wv, np.float32), np.asarray(bv, np.float32)
    Wo, bo = np.asarray(Wo, np.float32), np.asarray(bo, np.float32)

    nc = get_program()
    in_maps = _build_in_maps(x, Wq, bq, Wk, bk, Wv, Wo)
    res = run_bass_kernel_spmd(nc, in_maps, list(range(NCORES)))
    return _postprocess(res.results, Wo, bv, bo)

